# revision 1
# baseline (speedup 1.0000x reference)
"""Trainium2 Bass kernel for the 2-bit-DoReFa quantized BasicBlock.

  out = conv3x3(q(bn2(conv3x3(q(bn1(x)), Wq1))), Wq2) + x
  q(h) = round(3*clip(relu(h),0,1))/3,  Wq = DoReFa-2bit(w) in {-1,-1/3,1/3,1}

Strategy (data-parallel over batch, 4 images per NeuronCore x 8 cores):
  * Quantized activations/weights are exact small integers when scaled by 3:
    a3 in {0..3}, w3 in {-3,-1,1,3}.  Both are exact in bf16/fp8, and the
    3x3 convs become 18 (bf16) / 9 (fp8 DoubleRow) accumulating 128x128
    matmuls per output tile with *exact* integer accumulation in fp32 PSUM.
    The 1/9 rescale is folded into the next stage's BN scale.
  * Round-to-nearest-even is implemented exactly as (t + 2^23) - 2^23.
  * Conv reads a zero-padded 58-wide image laid out in SBUF; pixel tiles of
    464 columns (8 rows) keep each matmul in one PSUM bank.
"""
import os
from contextlib import ExitStack

import numpy as np

import concourse.bacc as bacc
import concourse.tile as tile
from concourse import mybir
from concourse.bass_utils import run_bass_kernel_spmd

F32 = mybir.dt.float32
OP = mybir.AluOpType
MAGIC = 8388608.0  # 2**23

N_CORES = 8
N_IMG = 4          # images per core (32 / 8)
C = 256
H = W = 56
PW = W + 2
NPIX = H * W
RT = 8
NT = H // RT
TQ = RT * PW       # 464
NPAD = ((PW * (H + 2) + 2 + 15) // 16) * 16

DOUBLE_ROW = True  # fp8 DoubleRow matmuls (exact for these integer operands)
ACT_DT = mybir.dt.float8e4 if DOUBLE_ROW else mybir.dt.bfloat16

LAST_EXEC_NS = None          # set when BASS_TRACE=1
_CACHED = {}


def _build():
    nc = bacc.Bacc("TRN2", target_bir_lowering=False, debug=False)

    x_d = nc.dram_tensor("x", [N_IMG, C, H, W], F32, kind="ExternalInput")
    w1_d = nc.dram_tensor("w1t", [128, 4608], ACT_DT, kind="ExternalInput")
    w2_d = nc.dram_tensor("w2t", [128, 4608], ACT_DT, kind="ExternalInput")
    prm_d = nc.dram_tensor("prm", [128, 8], F32, kind="ExternalInput")
    out_d = nc.dram_tensor("out", [N_IMG, C, H, W], F32, kind="ExternalOutput")

    xr = x_d.ap().rearrange("n (b k) h w -> n k b (h w)", b=2)
    outr = out_d.ap().rearrange("n (b k) h w -> n k b (h w)", b=2)

    with tile.TileContext(nc) as tc, ExitStack() as ctx:
        wpool = ctx.enter_context(tc.tile_pool(name="wpool", bufs=1))
        xpool = ctx.enter_context(tc.tile_pool(name="xpool", bufs=2))
        aqpool = ctx.enter_context(tc.tile_pool(name="aqpool", bufs=1))
        t1pool = ctx.enter_context(tc.tile_pool(name="t1pool", bufs=2))
        t2pool = ctx.enter_context(tc.tile_pool(name="t2pool", bufs=4))
        pspool = ctx.enter_context(tc.tile_pool(name="pspool", bufs=4,
                                                space="PSUM"))

        w1_sb = wpool.tile([128, 4608], ACT_DT)
        nc.sync.dma_start(w1_sb[:], w1_d.ap())
        w2_sb = wpool.tile([128, 4608], ACT_DT)
        nc.sync.dma_start(w2_sb[:], w2_d.ap())
        prm = wpool.tile([128, 8], F32)
        nc.sync.dma_start(prm[:], prm_d.ap())

        # Fixed ping-pong padded activation buffers; borders zeroed once.
        aq1s, aq2s = [], []
        for i in range(2):
            a1 = aqpool.tile([128, 2, NPAD], ACT_DT, name=f"aq1_{i}", tag=f"aq1_{i}")
            a2 = aqpool.tile([128, 2, NPAD], ACT_DT, name=f"aq2_{i}", tag=f"aq2_{i}")
            aq1s.append(a1)
            aq2s.append(a2)
            for a in (a1, a2):
                for blk in range(2):
                    nc.gpsimd.memset(a[:, blk, 0:PW + 1], 0.0)
                    mid = a[:, blk, PW + W + 1: PW + W + 1 + (H - 1) * PW]
                    mid3 = mid.rearrange("p (r c) -> p r c", c=PW)[:, :, 0:2]
                    nc.gpsimd.memset(mid3, 0.0)
                    nc.gpsimd.memset(a[:, blk, H * PW + W + 1: NPAD], 0.0)

        def quant_stage(src_ap, aq, blk, inv_col, bias_col, tmp_pool, rows, y0):
            # Replicates the reference fp32 op-for-op:
            #   t = x*inv + bias; t = relu(t); t = min(t,1)*3;
            #   t = (t + 2^23) - 2^23   (== round-to-nearest-even)
            t = tmp_pool.tile([128, rows * W], F32, tag="qtmp")
            t3 = t[:].rearrange("p (r c) -> p r c", c=W)
            nc.vector.tensor_scalar(t3, src_ap, prm[:, inv_col:inv_col + 1],
                                    prm[:, bias_col:bias_col + 1], OP.mult, OP.add)
            nc.scalar.activation(t3, t3, mybir.ActivationFunctionType.Relu)
            nc.vector.tensor_scalar(t3, t3, 1.0, 3.0, OP.min, OP.mult)
            dst = aq[:, blk, (y0 + 1) * PW + 1: (y0 + 1) * PW + 1 + rows * PW]
            dst3 = dst.rearrange("p (r c) -> p r c", c=PW)[:, :, 0:W]
            nc.vector.tensor_scalar(dst3, t3, MAGIC, MAGIC, OP.add, OP.subtract)

        def conv_tile(aq, w_sb, t, cb):
            ps = pspool.tile([128, TQ], F32, tag="ps")
            if DOUBLE_ROW:
                w4 = w_sb[:].rearrange("p (t j m) -> p t j m", t=9, j=2)
                for tap in range(9):
                    ky, kx = divmod(tap, 3)
                    lhsT = w4[:, tap, :, cb * 128:cb * 128 + 128]
                    rhs = aq[:, :, t * TQ + ky * PW + kx:
                             t * TQ + ky * PW + kx + TQ]
                    nc.tensor.matmul(ps[:], lhsT, rhs,
                                     perf_mode=mybir.MatmulPerfMode.DoubleRow,
                                     start=(tap == 0), stop=(tap == 8))
                return ps
            i = 0
            for blk in range(2):
                for tap in range(9):
                    ky, kx = divmod(tap, 3)
                    base = blk * 2304 + tap * 256 + cb * 128
                    rhs = aq[:, blk, t * TQ + ky * PW + kx:
                             t * TQ + ky * PW + kx + TQ]
                    nc.tensor.matmul(ps[:], w_sb[:, base:base + 128], rhs,
                                     start=(i == 0), stop=(i == 17))
                    i += 1
            return ps

        for img in range(N_IMG):
            aq1, aq2 = aq1s[img % 2], aq2s[img % 2]
            x_sb = xpool.tile([128, 2, NPIX], F32, tag="x")
            nc.sync.dma_start(x_sb[:], xr[img])

            for blk in range(2):
                src = x_sb[:, blk, :].rearrange("p (r c) -> p r c", c=W)
                quant_stage(src, aq1, blk, 0 + blk, 2 + blk, t1pool, H, 0)

            for t in range(NT):
                for cb in range(2):
                    ps = conv_tile(aq1, w1_sb, t, cb)
                    psv = ps[:].rearrange("p (r c) -> p r c", c=PW)[:, :, 0:W]
                    quant_stage(psv, aq2, cb, 4 + cb, 6 + cb, t2pool, RT, t * RT)

            for t in range(NT):
                for cb in range(2):
                    ps = conv_tile(aq2, w2_sb, t, cb)
                    psv = ps[:].rearrange("p (r c) -> p r c", c=PW)[:, :, 0:W]
                    res = x_sb[:, cb, t * RT * W: (t + 1) * RT * W]
                    res3 = res.rearrange("p (r c) -> p r c", c=W)
                    nc.vector.scalar_tensor_tensor(res3, psv, 1.0 / 9.0, res3,
                                                   OP.mult, OP.add)

            nc.sync.dma_start(outr[img], x_sb[:])

    nc.compile()
    return nc


def _host_prep(w1, w2, g1, b1, m1, v1, g2, b2, m2, v2):
    """BN folds + DoReFa weight quantization, replicating the reference's
    fp32 op sequence exactly (jax CPU), then weight layout transforms."""
    import jax
    import jax.numpy as jnp
    import ml_dtypes

    act_np = ml_dtypes.float8_e4m3 if DOUBLE_ROW else ml_dtypes.bfloat16
    cpu = jax.local_devices(backend="cpu")[0]
    with jax.default_device(cpu):
        eps = jnp.float32(1e-5)
        inv1 = g1 / jnp.sqrt(v1 + eps)
        bias1 = b1 - m1 * inv1
        inv2 = g2 / jnp.sqrt(v2 + eps)
        bias2 = b2 - m2 * inv2
        inv2_9 = inv2 / np.float32(9.0)

        def wq3(w):
            wt = jnp.tanh(w)
            wn = wt / (2.0 * jnp.max(jnp.abs(wt))) + 0.5
            return 2.0 * jnp.round(wn * 3.0) - 3.0   # exact ints {-3,-1,1,3}

        wq1 = np.asarray(wq3(jnp.asarray(w1)), dtype=np.float32)
        wq2 = np.asarray(wq3(jnp.asarray(w2)), dtype=np.float32)
        inv1, bias1, inv2_9, bias2 = (np.asarray(a, dtype=np.float32)
                                      for a in (inv1, bias1, inv2_9, bias2))

    def wlayout(wq):
        a = wq.reshape(256, 2, 128, 9)                     # cout, blk, k, tap
        perm = (2, 3, 1, 0) if DOUBLE_ROW else (2, 1, 3, 0)
        return np.ascontiguousarray(np.transpose(a, perm).reshape(128, 4608)
                                    ).astype(act_np)

    prm = np.zeros((128, 8), np.float32)
    for col, v in enumerate((inv1, bias1)):
        prm[:, 2 * col] = v[0:128]
        prm[:, 2 * col + 1] = v[128:256]
    for col, v in enumerate((inv2_9, bias2)):
        prm[:, 4 + 2 * col] = v[0:128]
        prm[:, 4 + 2 * col + 1] = v[128:256]

    return {"w1t": wlayout(wq1), "w2t": wlayout(wq2), "prm": prm}


def kernel(x, w1, w2, g1, b1, m1, v1, g2, b2, m2, v2):
    global LAST_EXEC_NS
    x = np.asarray(x, dtype=np.float32)

    if "nc" not in _CACHED:
        _CACHED["nc"] = _build()
    nc = _CACHED["nc"]

    shared = _host_prep(w1, w2, g1, b1, m1, v1, g2, b2, m2, v2)
    in_maps = []
    for c in range(N_CORES):
        m = dict(shared)
        m["x"] = x[N_IMG * c:N_IMG * (c + 1)]
        in_maps.append(m)

    trace = bool(int(os.environ.get("BASS_TRACE", "0")))
    res = run_bass_kernel_spmd(nc, in_maps, core_ids=list(range(N_CORES)),
                               trace=trace)
    LAST_EXEC_NS = res.exec_time_ns
    return np.concatenate([res.results[c]["out"] for c in range(N_CORES)],
                          axis=0)
